# revision 22
# baseline (speedup 1.0000x reference)
"""Tensor-parallel multi-head attention for Trainium2 (8 NeuronCores).

Problem: x:[2,16,2048,1024], wq/wk/wv:[64,1024], wo:[1024,1024]
  xq/xk/xv = einsum('bhsd,kd->bhsk', x, w)          (per-head, shared w)
  score    = xq @ xk.T / sqrt(1024); attn = softmax(score)
  out      = (attn @ xv) -> [B,S,H*dk] @ wo.T -> [B,S,1024]

Sharding: head-parallel over 8 cores (2 heads/core x 2 batches = 4
(b,h) pairs per core). Each core computes its heads' attention in
transposed layout (out.T blocks, rows = head dk), AllGathers the small
activation blocks (256KB/rank per pair, chunked and overlapped with
attention), then computes a 128-column slice of the output projection
(row-shard of wo.T) -- no all-reduce needed.

Performance structure:
- all matmuls bf16 (f32 PSUM accumulation)
- q/k projections merged into one M=128 matmul; q and k are then
  duplicated into both partition halves so score matmuls alternate
  32-row strips (concurrent row-tiled execution + hidden LDWEIGHTS)
- softmax denominator via an all-ones column appended to V (row 64 of
  the attn@V accumulator); normalization on the [64, S] output only
- exp batched as [128, 1024] ACT instructions straight from PSUM
- AllGather issued per (b,h) pair as soon as its block is ready;
  output projection for batch 0 overlaps batch 1's attention
"""

import os
import sys

import numpy as np

sys.path.insert(0, "/opt/trn_rl_repo")

import ml_dtypes  # noqa: E402

import concourse.bass as bass  # noqa: E402
import concourse.mybir as mybir  # noqa: E402
import concourse.tile as tile  # noqa: E402
from concourse import bacc  # noqa: E402
from concourse.bass_utils import run_bass_kernel_spmd  # noqa: E402
from concourse.masks import make_identity  # noqa: E402

N_CORES = 8
B, H, S, D = 2, 16, 2048, 1024
DK = D // H            # 64
HPC = H // N_CORES     # heads per core = 2
PAIRS = B * HPC        # (b, h) pairs per core = 4
SC = 512               # s-chunk (PSUM free-dim limit for f32)
NSC = S // SC          # 4 s-chunks per pair
NT = S // 128          # 16 t-tiles
NDC = D // 128         # 8 contraction chunks of 128
BS = B * S             # 4096 flattened (b, s) columns
INV_SCALE = 1.0 / 32.0  # 1/sqrt(D)

F32 = mybir.dt.float32
BF16 = mybir.dt.bfloat16

_GRAPH = None
LAST_RESULTS = None  # BassKernelResults of the most recent run (for test.py)


def _build_graph():
    nc = bacc.Bacc("TRN2", target_bir_lowering=False, num_devices=N_CORES)

    xt = nc.declare_dram_parameter("xt", [PAIRS, D, S], BF16, isOutput=False)
    wqk = nc.declare_dram_parameter("wqk", [D, 128], BF16, isOutput=False)
    wv = nc.declare_dram_parameter("wv", [D, DK], BF16, isOutput=False)
    wo = nc.declare_dram_parameter("wo", [D, 128], BF16, isOutput=False)
    out = nc.declare_dram_parameter("out", [128, BS], F32, isOutput=True)

    Exp = mybir.ActivationFunctionType.Exp

    with tile.TileContext(nc) as tc:
        with (
            tc.tile_pool(name="const", bufs=1) as cpool,
            tc.tile_pool(name="dram", bufs=1, space="DRAM") as dpool,
            tc.tile_pool(name="xin", bufs=2) as xpool,
            tc.tile_pool(name="qkv", bufs=2) as qkvpool,
            tc.tile_pool(name="vtiles", bufs=2) as vpool,
            tc.tile_pool(name="exp", bufs=3) as epool,
            tc.tile_pool(name="norm", bufs=2) as npool,
            tc.tile_pool(name="aio", bufs=1) as apool,
            tc.tile_pool(name="oout", bufs=2) as opool,
            tc.tile_pool(name="ps_proj", bufs=2, space="PSUM") as ps_proj,
            tc.tile_pool(name="ps_sc", bufs=2, space="PSUM") as ps_sc,
            tc.tile_pool(name="ps_ou", bufs=1, space="PSUM") as ps_ou,
        ):
            # Weights, bf16, laid out [128 partitions, chunk, m]
            wqk_sb = cpool.tile([128, NDC, 128], BF16)
            nc.sync.dma_start(
                out=wqk_sb[:], in_=wqk[:].rearrange("(c p) m -> p c m", p=128)
            )
            wv_sb = cpool.tile([128, NDC, DK], BF16)
            nc.sync.dma_start(
                out=wv_sb[:], in_=wv[:].rearrange("(c p) m -> p c m", p=128)
            )
            wo_sb = cpool.tile([128, NDC, 128], BF16)
            nc.sync.dma_start(
                out=wo_sb[:], in_=wo[:].rearrange("(c p) m -> p c m", p=128)
            )
            ident64 = cpool.tile([64, 64], BF16)
            make_identity(nc, ident64[:])

            # Collective bounce buffers: one chunk per (b, h) pair
            ag_in4 = dpool.tile([PAIRS, DK, S], BF16)
            ag_out4 = [
                dpool.tile(
                    [N_CORES, DK, S], BF16, addr_space="Shared",
                    name=f"ag_out_p{p}",
                )
                for p in range(PAIRS - 1)
            ]
            warm_in = dpool.tile([64, 16], BF16)
            warm_out = dpool.tile(
                [N_CORES, 64, 16], BF16, addr_space="Shared", name="warm_out"
            )
            # last pair gathered in two half-column chunks (shorter tail)
            ag_in_h = dpool.tile([2, DK, S // 2], BF16)
            ag_out_h = [
                dpool.tile(
                    [N_CORES, DK, S // 2], BF16, addr_space="Shared",
                    name=f"ag_out_h{g}",
                )
                for g in range(2)
            ]

            nc.vector.memset(warm_in_sb0 := cpool.tile([64, 16], BF16, name="warm_sb"), 0.0)
            nc.sync.dma_start(out=warm_in[:], in_=warm_in_sb0[:])
            nc.gpsimd.collective_compute(
                "AllGather",
                mybir.AluOpType.bypass,
                replica_groups=[list(range(N_CORES))],
                ins=[warm_in.opt()],
                outs=[warm_out.opt()],
            )

            asb_tiles = {}

            def outproj(b, ns, split=False, pool_tag=None):
                """Output projection for batch b's columns, s-chunks `ns`.
                split=True emits the lower 64 contraction rows (pair 2b,
                gathered earlier) as separate K=64 matmuls so they run
                before the upper rows' AllGather completes."""
                if b not in asb_tiles:
                    asb_tiles[b] = apool.tile(
                        [128, NDC, S], BF16, tag="asb", name=f"asb{b}"
                    )
                asb = asb_tiles[b]
                lo, hi = min(ns) * SC, (max(ns) + 1) * SC
                for c in range(NDC):
                    nc.sync.dma_start(
                        out=asb[0:64, c, lo:hi],
                        in_=ag_out4[HPC * b][c][:, lo:hi],
                    )
                    if HPC * b + 1 < PAIRS - 1:
                        nc.sync.dma_start(
                            out=asb[64:128, c, lo:hi],
                            in_=ag_out4[HPC * b + 1][c][:, lo:hi],
                        )
                    else:
                        g = lo // (S // 2)
                        h0 = g * (S // 2)
                        nc.sync.dma_start(
                            out=asb[64:128, c, lo:hi],
                            in_=ag_out_h[g][c][:, lo - h0 : hi - h0],
                        )
                oscope = nc.named_scope(f"outproj{b}_{min(ns)}")
                oscope.__enter__()
                pool, tag = (ps_sc, "sc") if pool_tag == "sc" else (
                    ps_proj, "proj_ps"
                )
                o_tiles = {}
                strips = ((0, 64), (64, 128)) if split else ((0, 128),)
                for si, (r0, r1) in enumerate(strips):
                    for n in ns:
                        if n not in o_tiles:
                            o_tiles[n] = pool.tile(
                                [128, SC], F32, tag=tag, name=f"o_ps{b}_{n}"
                            )
                        o_ps = o_tiles[n]
                        for c in range(NDC):
                            nc.tensor.matmul(
                                o_ps[:],
                                wo_sb[r0:r1, c, :],
                                asb[r0:r1, c, n * SC : (n + 1) * SC],
                                start=(c == 0 and si == 0),
                                stop=(
                                    c == NDC - 1 and si == len(strips) - 1
                                ),
                                tile_position=(r0, 0),
                            )
                for n in ns:
                    o_sb = opool.tile([128, SC], F32, tag="o_sb")
                    nc.vector.tensor_copy(o_sb[:], o_tiles[n][:])
                    nc.sync.dma_start(
                        out=out[:, b * S + n * SC : b * S + (n + 1) * SC],
                        in_=o_sb[:],
                    )
                oscope.__exit__(None, None, None)

            for p in range(PAIRS):
                b_idx = p // HPC
                hl = p % HPC
                import contextlib
                scope = nc.named_scope(f"prep{p}")
                scope.__enter__()

                # x.T for this pair: [128, chunk, S] bf16
                xT = xpool.tile([128, NDC, S], BF16, tag="xT")
                for c in range(NDC):
                    nc.sync.dma_start(
                        out=xT[:, c, :],
                        in_=xt[p][c * 128 : (c + 1) * 128, :],
                    )

                # merged q/k projection -> duplicated into both halves:
                # qd rows 0-63 = q.T, rows 64-127 = q.T (copy); kd likewise
                qd = qkvpool.tile([128, S], BF16, tag="qd")
                kd = qkvpool.tile([128, S], BF16, tag="kd")
                for n in range(NSC):
                    nsl = slice(n * SC, (n + 1) * SC)
                    ps_qk = ps_proj.tile([128, SC], F32, tag="proj_ps")
                    for c in range(NDC):
                        nc.tensor.matmul(
                            ps_qk[:],
                            wqk_sb[:, c, :],
                            xT[:, c, nsl],
                            start=(c == 0),
                            stop=(c == NDC - 1),
                        )
                    nc.vector.tensor_copy(qd[0:64, nsl], ps_qk[0:64, :])
                    nc.vector.tensor_copy(qd[64:128, nsl], ps_qk[0:64, :])
                    nc.vector.tensor_copy(kd[0:64, nsl], ps_qk[64:128, :])
                    nc.vector.tensor_copy(kd[64:128, nsl], ps_qk[64:128, :])

                # v projection + v' tiles, emitted in quarters and
                # software-pipelined into the first attention half below.
                vT_sb = qkvpool.tile([64, S], BF16, tag="vT")
                vt = [
                    vpool.tile([128, 65], BF16, tag=f"vt{t}", name=f"vt{t}")
                    for t in range(NT)
                ]

                def v_chain(j):
                    nsl = slice(j * SC, (j + 1) * SC)
                    ps_v = ps_proj.tile(
                        [64, SC], F32, tag="proj_ps", name="ps_v"
                    )
                    for c in range(NDC):
                        nc.tensor.matmul(
                            ps_v[:],
                            wv_sb[:, c, :],
                            xT[:, c, nsl],
                            start=(c == 0),
                            stop=(c == NDC - 1),
                        )
                    nc.vector.tensor_copy(vT_sb[:, nsl], ps_v[:])
                    vt_q = ps_proj.tile(
                        [128, 4, 64], BF16, tag="proj_ps", name="vt_q"
                    )
                    for tt in range(4 * j, 4 * j + 4):
                        nc.tensor.transpose(
                            vt_q[:, tt - 4 * j, :],
                            vT_sb[:, tt * 128 : (tt + 1) * 128],
                            ident64[:],
                        )
                        nc.vector.memset(vt[tt][:, 64:65], 1.0)
                        nc.vector.tensor_copy(
                            vt[tt][:, 0:64], vt_q[:, tt - 4 * j, :]
                        )

                v_chain(0)

                scope.__exit__(None, None, None)
                # Attention: two half-passes over s-chunks, t-outer.
                for half in range(NSC // 2):
                    if p == PAIRS - 1 and half == 1:
                        # batch-0 output projection fills PE idle slots of
                        # the ACT-bound attention stream
                        outproj(0, [0, 1, 2, 3])
                    ascope = nc.named_scope(f"attn{p}h{half}")
                    ascope.__enter__()
                    ou = []
                    for i in range(2):
                        ou_ps = ps_ou.tile(
                            [65, SC], F32, tag=f"ou{i}", name=f"ou{i}"
                        )
                        ou.append(ou_ps)
                    for t in range(NT):
                        if half == 0 and t % 4 == 0 and t // 4 < NSC - 1:
                            v_chain(t // 4 + 1)
                        tsl = slice(t * 128, (t + 1) * 128)
                        sc_big = ps_sc.tile([128, 2 * SC], F32, tag="sc")
                        pexp = epool.tile([128, 2 * SC], BF16, tag="pexp")
                        for i in range(2):
                            n = 2 * half + i
                            st = (n % 2) * 64
                            nc.tensor.matmul(
                                sc_big[:, i * SC : (i + 1) * SC],
                                kd[st : st + 64, tsl],
                                qd[st : st + 64, n * SC : (n + 1) * SC],
                                start=True,
                                stop=True,
                                tile_position=(st, 0),
                            )
                        nc.scalar.activation(
                            pexp[:], sc_big[:], Exp, scale=INV_SCALE
                        )
                        for i in range(2):
                            nc.tensor.matmul(
                                ou[i][:],
                                vt[t][:],
                                pexp[:, i * SC : (i + 1) * SC],
                                start=(t == 0),
                                stop=(t == NT - 1),
                            )
                    for i in range(2):
                        n = 2 * half + i
                        # copy accumulator to SBUF (frees the PSUM bank),
                        # then normalize rows 0..63 by row 64; reciprocal
                        # shifts partition 64 -> 0 (HW-verified)
                        ou_sb = npool.tile([65, SC], F32, tag="ou_sb")
                        nc.vector.tensor_copy(ou_sb[:], ou[i][:])
                        den0 = npool.tile([1, SC], F32, tag="den0")
                        nc.vector.tensor_copy(den0[0:1, :], ou_sb[64:65, :])
                        recip = npool.tile([1, SC], F32, tag="recip")
                        nc.vector.reciprocal_approx_fast(
                            recip[0:1, :], den0[0:1, :]
                        )
                        bcast = npool.tile([64, SC], F32, tag="bcast")
                        nc.gpsimd.partition_broadcast(bcast[:], recip[0:1, :])
                        onorm = npool.tile([64, SC], BF16, tag="onorm")
                        nc.vector.tensor_mul(
                            onorm[:], ou_sb[0:64, :], bcast[:]
                        )
                        if p < PAIRS - 1:
                            nc.sync.dma_start(
                                out=ag_in4[p][:, n * SC : (n + 1) * SC],
                                in_=onorm[:],
                            )
                        else:
                            nc.sync.dma_start(
                                out=ag_in_h[n // 2][
                                    :, (n % 2) * SC : (n % 2 + 1) * SC
                                ],
                                in_=onorm[:],
                            )
                    ascope.__exit__(None, None, None)
                    if p == PAIRS - 1:
                        # gather this half-column chunk immediately
                        nc.gpsimd.collective_compute(
                            "AllGather",
                            mybir.AluOpType.bypass,
                            replica_groups=[list(range(N_CORES))],
                            ins=[ag_in_h[half].opt()],
                            outs=[ag_out_h[half].opt()],
                        )

                # AllGather this pair's activation block (overlaps the
                # next pair's compute). The last pair was gathered in
                # half-column chunks inside the half-pass loop above.
                if p < PAIRS - 1:
                    nc.gpsimd.collective_compute(
                        "AllGather",
                        mybir.AluOpType.bypass,
                        replica_groups=[list(range(N_CORES))],
                        ins=[ag_in4[p].opt()],
                        outs=[ag_out4[p].opt()],
                    )

            outproj(1, [0, 1], split=True)
            outproj(1, [2, 3], split=True, pool_tag="sc")

    return nc


def _get_graph():
    global _GRAPH
    if _GRAPH is None:
        _GRAPH = _build_graph()
        if not _GRAPH.is_finalized():
            _GRAPH.finalize()
    return _GRAPH


def kernel(x, wq, wk, wv, wo):
    global LAST_RESULTS
    x = np.asarray(x, dtype=np.float32)
    wq = np.asarray(wq, dtype=np.float32)
    wk = np.asarray(wk, dtype=np.float32)
    wv = np.asarray(wv, dtype=np.float32)
    wo = np.asarray(wo, dtype=np.float32)

    bf16 = ml_dtypes.bfloat16
    # x transposed to [B, H, D, S] once (feeds matmuls as the moving operand)
    xt_all = np.ascontiguousarray(x.transpose(0, 1, 3, 2)).astype(bf16)
    wqk_t = np.ascontiguousarray(
        np.concatenate([wq, wk], axis=0).T
    ).astype(bf16)  # [D, 128]
    wv_t = np.ascontiguousarray(wv.T).astype(bf16)  # [D, 64]
    wo_t = np.ascontiguousarray(wo.T).astype(bf16)  # [D, D]; cols sliced per core

    in_maps = []
    for r in range(N_CORES):
        h0 = HPC * r
        # pair order: p = b*HPC + hl -> (b, h0+hl)
        xt_np = np.ascontiguousarray(
            xt_all[:, h0 : h0 + HPC].reshape(PAIRS, D, S)
        )
        in_maps.append(
            {
                "xt": xt_np,
                "wqk": wqk_t,
                "wv": wv_t,
                "wo": np.ascontiguousarray(wo_t[:, 128 * r : 128 * (r + 1)]),
            }
        )

    nc = _get_graph()
    trace = bool(os.environ.get("BASS_TRACE"))
    LAST_RESULTS = run_bass_kernel_spmd(
        nc, in_maps, core_ids=list(range(N_CORES)), trace=trace
    )
    outs = [LAST_RESULTS.results[r]["out"] for r in range(N_CORES)]
    full_t = np.concatenate(outs, axis=0)  # [D, B*S]
    return np.ascontiguousarray(full_t.T).reshape(B, S, D)


# revision 23
# speedup vs baseline: 1.0342x; 1.0342x over previous
"""Tensor-parallel multi-head attention for Trainium2 (8 NeuronCores).

Problem: x:[2,16,2048,1024], wq/wk/wv:[64,1024], wo:[1024,1024]
  xq/xk/xv = einsum('bhsd,kd->bhsk', x, w)          (per-head, shared w)
  score    = xq @ xk.T / sqrt(1024); attn = softmax(score)
  out      = (attn @ xv) -> [B,S,H*dk] @ wo.T -> [B,S,1024]

Sharding: head-parallel over 8 cores (2 heads/core x 2 batches = 4
(b,h) pairs per core). Each core computes its heads' attention in
transposed layout (out.T blocks, rows = head dk), AllGathers the small
activation blocks (256KB/rank per pair, chunked and overlapped with
attention), then computes a 128-column slice of the output projection
(row-shard of wo.T) -- no all-reduce needed.

Performance structure:
- all matmuls bf16 (f32 PSUM accumulation)
- q/k projections merged into one M=128 matmul; q and k are then
  duplicated into both partition halves so score matmuls alternate
  32-row strips (concurrent row-tiled execution + hidden LDWEIGHTS)
- softmax denominator via an all-ones column appended to V (row 64 of
  the attn@V accumulator); normalization on the [64, S] output only
- exp batched as [128, 1024] ACT instructions straight from PSUM
- AllGather issued per (b,h) pair as soon as its block is ready;
  output projection for batch 0 overlaps batch 1's attention
"""

import os
import sys

import numpy as np

sys.path.insert(0, "/opt/trn_rl_repo")

import ml_dtypes  # noqa: E402

import concourse.bass as bass  # noqa: E402
import concourse.mybir as mybir  # noqa: E402
import concourse.tile as tile  # noqa: E402
from concourse import bacc  # noqa: E402
from concourse.bass_utils import run_bass_kernel_spmd  # noqa: E402
from concourse.masks import make_identity  # noqa: E402

N_CORES = 8
B, H, S, D = 2, 16, 2048, 1024
DK = D // H            # 64
HPC = H // N_CORES     # heads per core = 2
PAIRS = B * HPC        # (b, h) pairs per core = 4
SC = 512               # s-chunk (PSUM free-dim limit for f32)
NSC = S // SC          # 4 s-chunks per pair
NT = S // 128          # 16 t-tiles
NDC = D // 128         # 8 contraction chunks of 128
BS = B * S             # 4096 flattened (b, s) columns
INV_SCALE = 1.0 / 32.0  # 1/sqrt(D)

F32 = mybir.dt.float32
BF16 = mybir.dt.bfloat16

_GRAPH = None
LAST_RESULTS = None  # BassKernelResults of the most recent run (for test.py)


def _build_graph():
    nc = bacc.Bacc("TRN2", target_bir_lowering=False, num_devices=N_CORES)

    xt = nc.declare_dram_parameter("xt", [PAIRS, D, S], BF16, isOutput=False)
    wqk = nc.declare_dram_parameter("wqk", [D, 128], BF16, isOutput=False)
    wv = nc.declare_dram_parameter("wv", [D, DK], BF16, isOutput=False)
    wo = nc.declare_dram_parameter("wo", [D, 128], BF16, isOutput=False)
    out = nc.declare_dram_parameter("out", [128, BS], F32, isOutput=True)

    Exp = mybir.ActivationFunctionType.Exp

    with tile.TileContext(nc) as tc:
        with (
            tc.tile_pool(name="const", bufs=1) as cpool,
            tc.tile_pool(name="dram", bufs=1, space="DRAM") as dpool,
            tc.tile_pool(name="xin", bufs=2) as xpool,
            tc.tile_pool(name="qkv", bufs=2) as qkvpool,
            tc.tile_pool(name="vtiles", bufs=2) as vpool,
            tc.tile_pool(name="exp", bufs=3) as epool,
            tc.tile_pool(name="norm", bufs=2) as npool,
            tc.tile_pool(name="aio", bufs=1) as apool,
            tc.tile_pool(name="oout", bufs=2) as opool,
            tc.tile_pool(name="ps_proj", bufs=2, space="PSUM") as ps_proj,
            tc.tile_pool(name="ps_sc", bufs=2, space="PSUM") as ps_sc,
            tc.tile_pool(name="ps_ou", bufs=1, space="PSUM") as ps_ou,
        ):
            # Weights, bf16, laid out [128 partitions, chunk, m]
            wqk_sb = cpool.tile([128, NDC, 128], BF16)
            nc.sync.dma_start(
                out=wqk_sb[:], in_=wqk[:].rearrange("(c p) m -> p c m", p=128)
            )
            wv_sb = cpool.tile([128, NDC, DK], BF16)
            nc.sync.dma_start(
                out=wv_sb[:], in_=wv[:].rearrange("(c p) m -> p c m", p=128)
            )
            wo_sb = cpool.tile([128, NDC, 128], BF16)
            nc.sync.dma_start(
                out=wo_sb[:], in_=wo[:].rearrange("(c p) m -> p c m", p=128)
            )
            ident64 = cpool.tile([64, 64], BF16)
            make_identity(nc, ident64[:])

            # Collective bounce buffers: one chunk per (b, h) pair
            ag_in4 = dpool.tile([PAIRS, DK, S], BF16)
            ag_out4 = [
                dpool.tile(
                    [N_CORES, DK, S], BF16, addr_space="Shared",
                    name=f"ag_out_p{p}",
                )
                for p in range(PAIRS - 1)
            ]
            warm_in = dpool.tile([64, 16], BF16)
            warm_out = dpool.tile(
                [N_CORES, 64, 16], BF16, addr_space="Shared", name="warm_out"
            )
            # last pair gathered in two half-column chunks (shorter tail)
            ag_in_h = dpool.tile([2, DK, S // 2], BF16)
            ag_out_h = [
                dpool.tile(
                    [N_CORES, DK, S // 2], BF16, addr_space="Shared",
                    name=f"ag_out_h{g}",
                )
                for g in range(2)
            ]

            nc.vector.memset(warm_in_sb0 := cpool.tile([64, 16], BF16, name="warm_sb"), 0.0)
            nc.sync.dma_start(out=warm_in[:], in_=warm_in_sb0[:])
            nc.gpsimd.collective_compute(
                "AllGather",
                mybir.AluOpType.bypass,
                replica_groups=[list(range(N_CORES))],
                ins=[warm_in.opt()],
                outs=[warm_out.opt()],
            )

            asb_tiles = {}

            def outproj(b, ns, split=False, pool_tag=None):
                """Output projection for batch b's columns, s-chunks `ns`.
                split=True emits the lower 64 contraction rows (pair 2b,
                gathered earlier) as separate K=64 matmuls so they run
                before the upper rows' AllGather completes."""
                if b not in asb_tiles:
                    asb_tiles[b] = apool.tile(
                        [128, NDC, S], BF16, tag="asb", name=f"asb{b}"
                    )
                asb = asb_tiles[b]
                lo, hi = min(ns) * SC, (max(ns) + 1) * SC
                for c in range(NDC):
                    nc.sync.dma_start(
                        out=asb[0:64, c, lo:hi],
                        in_=ag_out4[HPC * b][c][:, lo:hi],
                    )
                    if HPC * b + 1 < PAIRS - 1:
                        nc.sync.dma_start(
                            out=asb[64:128, c, lo:hi],
                            in_=ag_out4[HPC * b + 1][c][:, lo:hi],
                        )
                    else:
                        g = lo // (S // 2)
                        h0 = g * (S // 2)
                        nc.sync.dma_start(
                            out=asb[64:128, c, lo:hi],
                            in_=ag_out_h[g][c][:, lo - h0 : hi - h0],
                        )
                oscope = nc.named_scope(f"outproj{b}_{min(ns)}")
                oscope.__enter__()
                pool, tag = (ps_sc, "sc") if pool_tag == "sc" else (
                    ps_proj, "proj_ps"
                )
                o_tiles = {}
                strips = ((0, 64), (64, 128)) if split else ((0, 128),)
                for si, (r0, r1) in enumerate(strips):
                    for n in ns:
                        if n not in o_tiles:
                            o_tiles[n] = pool.tile(
                                [128, SC], F32, tag=tag, name=f"o_ps{b}_{n}"
                            )
                        o_ps = o_tiles[n]
                        for c in range(NDC):
                            nc.tensor.matmul(
                                o_ps[:],
                                wo_sb[r0:r1, c, :],
                                asb[r0:r1, c, n * SC : (n + 1) * SC],
                                start=(c == 0 and si == 0),
                                stop=(
                                    c == NDC - 1 and si == len(strips) - 1
                                ),
                                tile_position=(r0, 0),
                            )
                for n in ns:
                    o_sb = opool.tile([128, SC], F32, tag="o_sb")
                    nc.vector.tensor_copy(o_sb[:], o_tiles[n][:])
                    nc.sync.dma_start(
                        out=out[:, b * S + n * SC : b * S + (n + 1) * SC],
                        in_=o_sb[:],
                    )
                oscope.__exit__(None, None, None)

            for p in range(PAIRS):
                b_idx = p // HPC
                hl = p % HPC
                import contextlib
                scope = nc.named_scope(f"prep{p}")
                scope.__enter__()

                # x.T for this pair: [128, chunk, S] bf16
                xT = xpool.tile([128, NDC, S], BF16, tag="xT")
                for c in range(NDC):
                    nc.sync.dma_start(
                        out=xT[:, c, :],
                        in_=xt[p][c * 128 : (c + 1) * 128, :],
                    )

                # merged q/k projection -> duplicated into both halves:
                # qd rows 0-63 = q.T, rows 64-127 = q.T (copy); kd likewise
                qd = qkvpool.tile([128, S], BF16, tag="qd")
                kd = qkvpool.tile([128, S], BF16, tag="kd")
                for n in range(NSC):
                    nsl = slice(n * SC, (n + 1) * SC)
                    ps_qk = ps_proj.tile([128, SC], F32, tag="proj_ps")
                    for c in range(NDC):
                        nc.tensor.matmul(
                            ps_qk[:],
                            wqk_sb[:, c, :],
                            xT[:, c, nsl],
                            start=(c == 0),
                            stop=(c == NDC - 1),
                        )
                    nc.vector.tensor_copy(qd[0:64, nsl], ps_qk[0:64, :])
                    nc.vector.tensor_copy(qd[64:128, nsl], ps_qk[0:64, :])
                    nc.vector.tensor_copy(kd[0:64, nsl], ps_qk[64:128, :])
                    nc.vector.tensor_copy(kd[64:128, nsl], ps_qk[64:128, :])

                # v projection + v' tiles, emitted in quarters and
                # software-pipelined into the first attention half below.
                vT_sb = qkvpool.tile([64, S], BF16, tag="vT")
                vt = [
                    vpool.tile([128, 65], BF16, tag=f"vt{t}", name=f"vt{t}")
                    for t in range(NT)
                ]

                def v_chain(j):
                    nsl = slice(j * SC, (j + 1) * SC)
                    ps_v = ps_proj.tile(
                        [64, SC], F32, tag="proj_ps", name="ps_v"
                    )
                    for c in range(NDC):
                        nc.tensor.matmul(
                            ps_v[:],
                            wv_sb[:, c, :],
                            xT[:, c, nsl],
                            start=(c == 0),
                            stop=(c == NDC - 1),
                        )
                    nc.vector.tensor_copy(vT_sb[:, nsl], ps_v[:])
                    vt_q = ps_proj.tile(
                        [128, 4, 64], BF16, tag="proj_ps", name="vt_q"
                    )
                    for tt in range(4 * j, 4 * j + 4):
                        nc.tensor.transpose(
                            vt_q[:, tt - 4 * j, :],
                            vT_sb[:, tt * 128 : (tt + 1) * 128],
                            ident64[:],
                        )
                        nc.vector.memset(vt[tt][:, 64:65], 1.0)
                        nc.vector.tensor_copy(
                            vt[tt][:, 0:64], vt_q[:, tt - 4 * j, :]
                        )

                v_chain(0)

                scope.__exit__(None, None, None)
                # Attention: two half-passes over s-chunks, t-outer.
                for half in range(NSC // 2):
                    if p == PAIRS - 1 and half == 1:
                        # batch-0 output projection fills PE idle slots of
                        # the ACT-bound attention stream
                        outproj(0, [0, 1, 2, 3])
                    ascope = nc.named_scope(f"attn{p}h{half}")
                    ascope.__enter__()
                    ou = []
                    for i in range(2):
                        ou_ps = ps_ou.tile(
                            [65, SC], F32, tag=f"ou{i}", name=f"ou{i}"
                        )
                        ou.append(ou_ps)
                    for t in range(NT):
                        if half == 0 and t % 4 == 0 and t // 4 < NSC - 1:
                            v_chain(t // 4 + 1)
                        tsl = slice(t * 128, (t + 1) * 128)
                        sc_big = ps_sc.tile([128, 2 * SC], F32, tag="sc")
                        pexp = epool.tile([128, 2 * SC], BF16, tag="pexp")
                        for i in range(2):
                            n = 2 * half + i
                            st = (n % 2) * 64
                            nc.tensor.matmul(
                                sc_big[:, i * SC : (i + 1) * SC],
                                kd[st : st + 64, tsl],
                                qd[st : st + 64, n * SC : (n + 1) * SC],
                                start=True,
                                stop=True,
                                tile_position=(st, 0),
                            )
                        nc.scalar.activation(
                            pexp[:], sc_big[:], Exp, scale=INV_SCALE
                        )
                        for i in range(2):
                            nc.tensor.matmul(
                                ou[i][:],
                                vt[t][:],
                                pexp[:, i * SC : (i + 1) * SC],
                                start=(t == 0),
                                stop=(t == NT - 1),
                            )
                    for i in range(2):
                        n = 2 * half + i
                        # copy accumulator to SBUF (frees the PSUM bank),
                        # then normalize rows 0..63 by row 64; reciprocal
                        # shifts partition 64 -> 0 (HW-verified)
                        ou_sb = npool.tile([65, SC], F32, tag="ou_sb")
                        nc.vector.tensor_copy(ou_sb[:], ou[i][:])
                        den0 = npool.tile([1, SC], F32, tag="den0")
                        nc.vector.tensor_copy(den0[0:1, :], ou_sb[64:65, :])
                        recip = npool.tile([1, SC], F32, tag="recip")
                        nc.vector.reciprocal_approx_fast(
                            recip[0:1, :], den0[0:1, :]
                        )
                        bcast = npool.tile([64, SC], F32, tag="bcast")
                        nc.gpsimd.partition_broadcast(bcast[:], recip[0:1, :])
                        onorm = npool.tile([64, SC], BF16, tag="onorm")
                        nc.vector.tensor_mul(
                            onorm[:], ou_sb[0:64, :], bcast[:]
                        )
                        if p < PAIRS - 1:
                            nc.sync.dma_start(
                                out=ag_in4[p][:, n * SC : (n + 1) * SC],
                                in_=onorm[:],
                            )
                        else:
                            nc.sync.dma_start(
                                out=ag_in_h[n // 2][
                                    :, (n % 2) * SC : (n % 2 + 1) * SC
                                ],
                                in_=onorm[:],
                            )
                    ascope.__exit__(None, None, None)
                    if p == PAIRS - 1:
                        # gather this half-column chunk immediately
                        nc.gpsimd.collective_compute(
                            "AllGather",
                            mybir.AluOpType.bypass,
                            replica_groups=[list(range(N_CORES))],
                            ins=[ag_in_h[half].opt()],
                            outs=[ag_out_h[half].opt()],
                        )

                # AllGather this pair's activation block (overlaps the
                # next pair's compute). The last pair was gathered in
                # half-column chunks inside the half-pass loop above.
                if p < PAIRS - 1:
                    nc.gpsimd.collective_compute(
                        "AllGather",
                        mybir.AluOpType.bypass,
                        replica_groups=[list(range(N_CORES))],
                        ins=[ag_in4[p].opt()],
                        outs=[ag_out4[p].opt()],
                    )

            outproj(1, [0, 1], split=True)
            outproj(1, [2, 3], split=True, pool_tag="sc")

    return nc


def _get_graph():
    global _GRAPH
    if _GRAPH is None:
        _GRAPH = _build_graph()
        if not _GRAPH.is_finalized():
            _GRAPH.finalize()
    return _GRAPH


def kernel(x, wq, wk, wv, wo):
    global LAST_RESULTS
    x = np.asarray(x, dtype=np.float32)
    wq = np.asarray(wq, dtype=np.float32)
    wk = np.asarray(wk, dtype=np.float32)
    wv = np.asarray(wv, dtype=np.float32)
    wo = np.asarray(wo, dtype=np.float32)

    bf16 = ml_dtypes.bfloat16
    # x transposed to [B, H, D, S] once (feeds matmuls as the moving operand)
    xt_all = np.ascontiguousarray(x.transpose(0, 1, 3, 2)).astype(bf16)
    wqk_t = np.ascontiguousarray(
        np.concatenate([wq, wk], axis=0).T
    ).astype(bf16)  # [D, 128]
    wv_t = np.ascontiguousarray(wv.T).astype(bf16)  # [D, 64]
    wo_t = np.ascontiguousarray(wo.T).astype(bf16)  # [D, D]; cols sliced per core

    in_maps = []
    for r in range(N_CORES):
        h0 = HPC * r
        # pair order: p = b*HPC + hl -> (b, h0+hl)
        xt_np = np.ascontiguousarray(
            xt_all[:, h0 : h0 + HPC].reshape(PAIRS, D, S)
        )
        in_maps.append(
            {
                "xt": xt_np,
                "wqk": wqk_t,
                "wv": wv_t,
                "wo": np.ascontiguousarray(wo_t[:, 128 * r : 128 * (r + 1)]),
            }
        )

    nc = _get_graph()
    trace = bool(os.environ.get("BASS_TRACE"))
    tk = {}
    tc_env = os.environ.get("TRACE_CORES")
    if tc_env:
        tk["trace_cores"] = [int(c) for c in tc_env.split(",")]
    LAST_RESULTS = run_bass_kernel_spmd(
        nc, in_maps, core_ids=list(range(N_CORES)), trace=trace, **tk
    )
    outs = [LAST_RESULTS.results[r]["out"] for r in range(N_CORES)]
    full_t = np.concatenate(outs, axis=0)  # [D, B*S]
    return np.ascontiguousarray(full_t.T).reshape(B, S, D)


# revision 30
# speedup vs baseline: 1.0898x; 1.0537x over previous
"""Tensor-parallel multi-head attention for Trainium2 (8 NeuronCores).

Problem: x:[2,16,2048,1024], wq/wk/wv:[64,1024], wo:[1024,1024]
  xq/xk/xv = einsum('bhsd,kd->bhsk', x, w)          (per-head, shared w)
  score    = xq @ xk.T / sqrt(1024); attn = softmax(score)
  out      = (attn @ xv) -> [B,S,H*dk] @ wo.T -> [B,S,1024]

Sharding: head-parallel over 8 cores (2 heads/core x 2 batches = 4
(b,h) pairs per core). Each core computes its heads' attention in
transposed layout (out.T blocks, rows = head dk), AllGathers the small
activation blocks (256KB/rank per pair, chunked and overlapped with
attention), then computes a 128-column slice of the output projection
(row-shard of wo.T) -- no all-reduce needed.

Performance structure:
- all matmuls bf16 (f32 PSUM accumulation)
- q/k projections merged into one M=128 matmul; q and k are then
  duplicated into both partition halves so score matmuls alternate
  32-row strips (concurrent row-tiled execution + hidden LDWEIGHTS)
- softmax denominator via an all-ones column appended to V (row 64 of
  the attn@V accumulator); normalization on the [64, S] output only
- exp batched as [128, 1024] ACT instructions straight from PSUM
- AllGather issued per (b,h) pair as soon as its block is ready;
  output projection for batch 0 overlaps batch 1's attention
"""

import os
import sys

import numpy as np

sys.path.insert(0, "/opt/trn_rl_repo")

import ml_dtypes  # noqa: E402

import concourse.bass as bass  # noqa: E402
import concourse.mybir as mybir  # noqa: E402
import concourse.tile as tile  # noqa: E402
from concourse import bacc  # noqa: E402
from concourse.bass_utils import run_bass_kernel_spmd  # noqa: E402
from concourse.masks import make_identity  # noqa: E402

N_CORES = 8
B, H, S, D = 2, 16, 2048, 1024
DK = D // H            # 64
HPC = H // N_CORES     # heads per core = 2
PAIRS = B * HPC        # (b, h) pairs per core = 4
SC = 512               # s-chunk (PSUM free-dim limit for f32)
NSC = S // SC          # 4 s-chunks per pair
NT = S // 128          # 16 t-tiles
NDC = D // 128         # 8 contraction chunks of 128
BS = B * S             # 4096 flattened (b, s) columns
INV_SCALE = 1.0 / 32.0  # 1/sqrt(D)

F32 = mybir.dt.float32
BF16 = mybir.dt.bfloat16

_GRAPH = None
LAST_RESULTS = None  # BassKernelResults of the most recent run (for test.py)


def _build_graph():
    nc = bacc.Bacc("TRN2", target_bir_lowering=False, num_devices=N_CORES)

    xt = nc.declare_dram_parameter("xt", [PAIRS, D, S], BF16, isOutput=False)
    wqk = nc.declare_dram_parameter("wqk", [D, 128], BF16, isOutput=False)
    wv = nc.declare_dram_parameter("wv", [D, DK], BF16, isOutput=False)
    wo = nc.declare_dram_parameter("wo", [D, 128], BF16, isOutput=False)
    out = nc.declare_dram_parameter("out", [128, BS], F32, isOutput=True)

    Exp = mybir.ActivationFunctionType.Exp

    with tile.TileContext(nc) as tc:
        with (
            tc.tile_pool(name="const", bufs=1) as cpool,
            tc.tile_pool(name="dram", bufs=1, space="DRAM") as dpool,
            tc.tile_pool(name="xin", bufs=2) as xpool,
            tc.tile_pool(name="qkv", bufs=2) as qkvpool,
            tc.tile_pool(name="vtiles", bufs=2) as vpool,
            tc.tile_pool(name="exp", bufs=3) as epool,
            tc.tile_pool(name="norm", bufs=2) as npool,
            tc.tile_pool(name="aio", bufs=1) as apool,
            tc.tile_pool(name="oout", bufs=2) as opool,
            tc.tile_pool(name="ps_proj", bufs=2, space="PSUM") as ps_proj,
            tc.tile_pool(name="ps_sc", bufs=2, space="PSUM") as ps_sc,
            tc.tile_pool(name="ps_ou", bufs=1, space="PSUM") as ps_ou,
        ):
            # Weights, bf16, laid out [128 partitions, chunk, m]
            wqk_sb = cpool.tile([128, NDC, 128], BF16)
            nc.sync.dma_start(
                out=wqk_sb[:], in_=wqk[:].rearrange("(c p) m -> p c m", p=128)
            )
            wv_sb = cpool.tile([128, NDC, DK], BF16)
            nc.sync.dma_start(
                out=wv_sb[:], in_=wv[:].rearrange("(c p) m -> p c m", p=128)
            )
            wo_sb = cpool.tile([128, NDC, 128], BF16)
            nc.sync.dma_start(
                out=wo_sb[:], in_=wo[:].rearrange("(c p) m -> p c m", p=128)
            )
            ident64 = cpool.tile([64, 64], BF16)
            make_identity(nc, ident64[:])

            # Collective bounce buffers: one chunk per (b, h) pair
            ag_in4 = dpool.tile([PAIRS, DK, S], BF16)
            ag_out4 = [
                dpool.tile(
                    [N_CORES, DK, S], BF16, addr_space="Shared",
                    name=f"ag_out_p{p}",
                )
                for p in range(PAIRS - 1)
            ]
            warm_in = dpool.tile([64, 16], BF16)
            warm_out = dpool.tile(
                [N_CORES, 64, 16], BF16, addr_space="Shared", name="warm_out"
            )
            # last pair gathered in two half-column chunks (shorter tail)
            ag_in_h = dpool.tile([2, DK, S // 2], BF16)
            ag_out_h = [
                dpool.tile(
                    [N_CORES, DK, S // 2], BF16, addr_space="Shared",
                    name=f"ag_out_h{g}",
                )
                for g in range(2)
            ]

            nc.vector.memset(warm_in_sb0 := cpool.tile([64, 16], BF16, name="warm_sb"), 0.0)
            nc.sync.dma_start(out=warm_in[:], in_=warm_in_sb0[:])
            nc.gpsimd.collective_compute(
                "AllGather",
                mybir.AluOpType.bypass,
                replica_groups=[list(range(N_CORES))],
                ins=[warm_in.opt()],
                outs=[warm_out.opt()],
            )

            asb_tiles = {}

            def get_asb(b):
                if b not in asb_tiles:
                    asb_tiles[b] = apool.tile(
                        [128, NDC, S], BF16, tag="asb", name=f"asb{b}"
                    )
                return asb_tiles[b]

            def outproj_dma(b, ns, rows=("lower", "upper")):
                """Load the gathered activation rows for batch b's columns.
                Lower rows (pair 2b) are available as soon as that pair's
                AllGather lands, so they can be prefetched early."""
                asb = get_asb(b)
                lo, hi = min(ns) * SC, (max(ns) + 1) * SC
                for c in range(NDC):
                    if "lower" in rows:
                        nc.sync.dma_start(
                            out=asb[0:64, c, lo:hi],
                            in_=ag_out4[HPC * b][c][:, lo:hi],
                        )
                    if "upper" not in rows:
                        continue
                    if HPC * b + 1 < PAIRS - 1:
                        nc.sync.dma_start(
                            out=asb[64:128, c, lo:hi],
                            in_=ag_out4[HPC * b + 1][c][:, lo:hi],
                        )
                    else:
                        g = lo // (S // 2)
                        h0 = g * (S // 2)
                        nc.sync.dma_start(
                            out=asb[64:128, c, lo:hi],
                            in_=ag_out_h[g][c][:, lo - h0 : hi - h0],
                        )

            def outproj(b, ns, split=False, pool_tag=None, do_dma=True):
                """Output projection for batch b's columns, s-chunks `ns`.
                split=True emits the lower 64 contraction rows as separate
                K=64 matmuls so they run before the upper rows' AllGather
                completes."""
                if do_dma:
                    outproj_dma(b, ns)
                asb = get_asb(b)
                oscope = nc.named_scope(f"outproj{b}_{min(ns)}")
                oscope.__enter__()
                pool, tag = (ps_sc, "sc") if pool_tag == "sc" else (
                    ps_proj, "proj_ps"
                )
                o_tiles = {}
                strips = ((0, 64), (64, 128)) if split else ((0, 128),)
                for si, (r0, r1) in enumerate(strips):
                    for n in ns:
                        if n not in o_tiles:
                            o_tiles[n] = pool.tile(
                                [128, SC], F32, tag=tag, name=f"o_ps{b}_{n}"
                            )
                        o_ps = o_tiles[n]
                        for c in range(NDC):
                            nc.tensor.matmul(
                                o_ps[:],
                                wo_sb[r0:r1, c, :],
                                asb[r0:r1, c, n * SC : (n + 1) * SC],
                                start=(c == 0 and si == 0),
                                stop=(
                                    c == NDC - 1 and si == len(strips) - 1
                                ),
                                tile_position=(r0, 0),
                            )
                for n in ns:
                    o_sb = opool.tile([128, SC], F32, tag="o_sb")
                    nc.vector.tensor_copy(o_sb[:], o_tiles[n][:])
                    nc.sync.dma_start(
                        out=out[:, b * S + n * SC : b * S + (n + 1) * SC],
                        in_=o_sb[:],
                    )
                oscope.__exit__(None, None, None)

            def emit_xT(p):
                xT = xpool.tile([128, NDC, S], BF16, tag="xT", name=f"xT{p}")
                for c in range(NDC):
                    nc.sync.dma_start(
                        out=xT[:, c, :],
                        in_=xt[p][c * 128 : (c + 1) * 128, :],
                    )
                return xT

            def emit_qk_group(xT, qd, kd, n):
                """One n-chunk of the merged q/k projection, with q and k
                duplicated into both partition halves for strip-alternating
                score matmuls."""
                nsl = slice(n * SC, (n + 1) * SC)
                ps_qk = ps_proj.tile([128, SC], F32, tag="proj_ps", name="ps_qk")
                for c in range(NDC):
                    nc.tensor.matmul(
                        ps_qk[:],
                        wqk_sb[:, c, :],
                        xT[:, c, nsl],
                        start=(c == 0),
                        stop=(c == NDC - 1),
                    )
                nc.vector.tensor_copy(qd[0:64, nsl], ps_qk[0:64, :])
                nc.vector.tensor_copy(qd[64:128, nsl], ps_qk[0:64, :])
                nc.vector.tensor_copy(kd[0:64, nsl], ps_qk[64:128, :])
                nc.vector.tensor_copy(kd[64:128, nsl], ps_qk[64:128, :])

            def alloc_qdkd(p):
                qd = qkvpool.tile([128, S], BF16, tag="qd", name=f"qd{p}")
                kd = qkvpool.tile([128, S], BF16, tag="kd", name=f"kd{p}")
                return qd, kd

            # software pipeline: pair 0's projections run up front; pair
            # p+1's x load + q/k projection are emitted inside pair p's
            # second (ACT-bound) attention half to fill PE idle slots.
            xT_t = {0: emit_xT(0)}
            qdkd = {0: alloc_qdkd(0)}
            for n in range(NSC):
                emit_qk_group(xT_t[0], *qdkd[0], n)

            for p in range(PAIRS):
                b_idx = p // HPC
                hl = p % HPC
                xT = xT_t.pop(p)
                qd, kd = qdkd.pop(p)

                # v projection + v' tiles, emitted in quarters and
                # software-pipelined into the first attention half below.
                vT_sb = qkvpool.tile([64, S], BF16, tag="vT")
                vt = [
                    vpool.tile([128, 65], BF16, tag=f"vt{t}", name=f"vt{t}")
                    for t in range(NT)
                ]

                def v_chain(j, xT=xT, vT_sb=vT_sb, vt=vt):
                    nsl = slice(j * SC, (j + 1) * SC)
                    ps_v = ps_proj.tile(
                        [64, SC], F32, tag="proj_ps", name="ps_v"
                    )
                    for c in range(NDC):
                        nc.tensor.matmul(
                            ps_v[:],
                            wv_sb[:, c, :],
                            xT[:, c, nsl],
                            start=(c == 0),
                            stop=(c == NDC - 1),
                        )
                    nc.vector.tensor_copy(vT_sb[:, nsl], ps_v[:])
                    vt_q = ps_proj.tile(
                        [128, 4, 64], BF16, tag="proj_ps", name="vt_q"
                    )
                    for tt in range(4 * j, 4 * j + 4):
                        nc.tensor.transpose(
                            vt_q[:, tt - 4 * j, :],
                            vT_sb[:, tt * 128 : (tt + 1) * 128],
                            ident64[:],
                        )
                        nc.vector.memset(vt[tt][:, 64:65], 1.0)
                        nc.vector.tensor_copy(
                            vt[tt][:, 0:64], vt_q[:, tt - 4 * j, :]
                        )

                v_chain(0)
                if p + 1 < PAIRS:
                    qdkd[p + 1] = alloc_qdkd(p + 1)

                # Attention: two half-passes over s-chunks, t-outer.
                for half in range(NSC // 2):
                    if p == 2 and half == 1:
                        # prefetch batch-0 activations (AG0/AG1 long done)
                        outproj_dma(0, [0, 1, 2, 3])
                    if p == PAIRS - 1 and half == 1:
                        # prefetch batch-1 lower rows (AG2 has landed) and
                        # run the batch-0 projection in PE idle slots of
                        # the ACT-bound attention stream
                        outproj_dma(1, [0, 1, 2, 3], rows=("lower",))
                        outproj(0, [0, 1, 2, 3], do_dma=False)
                    ascope = nc.named_scope(f"attn{p}h{half}")
                    ascope.__enter__()
                    ou = []
                    for i in range(2):
                        ou_ps = ps_ou.tile(
                            [65, SC], F32, tag=f"ou{i}", name=f"ou{i}"
                        )
                        ou.append(ou_ps)
                    for t in range(NT):
                        if half == 0 and t % 4 == 0 and t // 4 < NSC - 1:
                            v_chain(t // 4 + 1)
                        if half == 0 and t == 8 and p + 1 < PAIRS:
                            # x load for the next pair, placed after the
                            # previous pair's AllGather window has drained
                            xT_t[p + 1] = emit_xT(p + 1)
                        if half == 1 and t % 4 == 0 and p + 1 < PAIRS:
                            emit_qk_group(
                                xT_t[p + 1], *qdkd[p + 1], t // 4
                            )
                        tsl = slice(t * 128, (t + 1) * 128)
                        sc_big = ps_sc.tile([128, 2 * SC], F32, tag="sc")
                        pexp = epool.tile([128, 2 * SC], BF16, tag="pexp")
                        for i in range(2):
                            n = 2 * half + i
                            st = (n % 2) * 64
                            nc.tensor.matmul(
                                sc_big[:, i * SC : (i + 1) * SC],
                                kd[st : st + 64, tsl],
                                qd[st : st + 64, n * SC : (n + 1) * SC],
                                start=True,
                                stop=True,
                                tile_position=(st, 0),
                            )
                        nc.scalar.activation(
                            pexp[:], sc_big[:], Exp, scale=INV_SCALE
                        )
                        for i in range(2):
                            nc.tensor.matmul(
                                ou[i][:],
                                vt[t][:],
                                pexp[:, i * SC : (i + 1) * SC],
                                start=(t == 0),
                                stop=(t == NT - 1),
                            )
                    for i in range(2):
                        n = 2 * half + i
                        # copy accumulator to SBUF (frees the PSUM bank),
                        # then normalize rows 0..63 by row 64; reciprocal
                        # shifts partition 64 -> 0 (HW-verified)
                        ou_sb = npool.tile([65, SC], F32, tag="ou_sb")
                        nc.vector.tensor_copy(ou_sb[:], ou[i][:])
                        den0 = npool.tile([1, SC], F32, tag="den0")
                        nc.vector.tensor_copy(den0[0:1, :], ou_sb[64:65, :])
                        recip = npool.tile([1, SC], F32, tag="recip")
                        nc.vector.reciprocal_approx_fast(
                            recip[0:1, :], den0[0:1, :]
                        )
                        bcast = npool.tile([64, SC], F32, tag="bcast")
                        nc.gpsimd.partition_broadcast(bcast[:], recip[0:1, :])
                        onorm = npool.tile([64, SC], BF16, tag="onorm")
                        nc.vector.tensor_mul(
                            onorm[:], ou_sb[0:64, :], bcast[:]
                        )
                        if p < PAIRS - 1:
                            nc.sync.dma_start(
                                out=ag_in4[p][:, n * SC : (n + 1) * SC],
                                in_=onorm[:],
                            )
                        else:
                            nc.sync.dma_start(
                                out=ag_in_h[n // 2][
                                    :, (n % 2) * SC : (n % 2 + 1) * SC
                                ],
                                in_=onorm[:],
                            )
                    ascope.__exit__(None, None, None)
                    if p == PAIRS - 1:
                        # gather this half-column chunk immediately
                        nc.gpsimd.collective_compute(
                            "AllGather",
                            mybir.AluOpType.bypass,
                            replica_groups=[list(range(N_CORES))],
                            ins=[ag_in_h[half].opt()],
                            outs=[ag_out_h[half].opt()],
                        )

                # AllGather this pair's activation block (overlaps the
                # next pair's compute). The last pair was gathered in
                # half-column chunks inside the half-pass loop above.
                if p < PAIRS - 1:
                    nc.gpsimd.collective_compute(
                        "AllGather",
                        mybir.AluOpType.bypass,
                        replica_groups=[list(range(N_CORES))],
                        ins=[ag_in4[p].opt()],
                        outs=[ag_out4[p].opt()],
                    )

            outproj_dma(1, [0, 1], rows=("upper",))
            outproj(1, [0, 1], split=True, do_dma=False)
            outproj_dma(1, [2, 3], rows=("upper",))
            outproj(1, [2, 3], split=True, pool_tag="sc", do_dma=False)

    return nc


def _get_graph():
    global _GRAPH
    if _GRAPH is None:
        _GRAPH = _build_graph()
        if not _GRAPH.is_finalized():
            _GRAPH.finalize()
    return _GRAPH


def kernel(x, wq, wk, wv, wo):
    global LAST_RESULTS
    x = np.asarray(x, dtype=np.float32)
    wq = np.asarray(wq, dtype=np.float32)
    wk = np.asarray(wk, dtype=np.float32)
    wv = np.asarray(wv, dtype=np.float32)
    wo = np.asarray(wo, dtype=np.float32)

    bf16 = ml_dtypes.bfloat16
    # x transposed to [B, H, D, S] once (feeds matmuls as the moving operand)
    xt_all = np.ascontiguousarray(x.transpose(0, 1, 3, 2)).astype(bf16)
    wqk_t = np.ascontiguousarray(
        np.concatenate([wq, wk], axis=0).T
    ).astype(bf16)  # [D, 128]
    wv_t = np.ascontiguousarray(wv.T).astype(bf16)  # [D, 64]
    wo_t = np.ascontiguousarray(wo.T).astype(bf16)  # [D, D]; cols sliced per core

    in_maps = []
    for r in range(N_CORES):
        h0 = HPC * r
        # pair order: p = b*HPC + hl -> (b, h0+hl)
        xt_np = np.ascontiguousarray(
            xt_all[:, h0 : h0 + HPC].reshape(PAIRS, D, S)
        )
        in_maps.append(
            {
                "xt": xt_np,
                "wqk": wqk_t,
                "wv": wv_t,
                "wo": np.ascontiguousarray(wo_t[:, 128 * r : 128 * (r + 1)]),
            }
        )

    nc = _get_graph()
    trace = bool(os.environ.get("BASS_TRACE"))
    if trace:
        try:  # tracing needs the axon NTFF hook; fall back cleanly
            from antenv.axon_hooks import get_axon_ntff_profile_hook  # noqa: F401
        except ImportError:
            trace = False
    tk = {}
    tc_env = os.environ.get("TRACE_CORES")
    if tc_env:
        tk["trace_cores"] = [int(c) for c in tc_env.split(",")]
    LAST_RESULTS = run_bass_kernel_spmd(
        nc, in_maps, core_ids=list(range(N_CORES)), trace=trace, **tk
    )
    outs = [LAST_RESULTS.results[r]["out"] for r in range(N_CORES)]
    full_t = np.concatenate(outs, axis=0)  # [D, B*S]
    return np.ascontiguousarray(full_t.T).reshape(B, S, D)
